# revision 1
# baseline (speedup 1.0000x reference)
"""Bass/Trainium2 kernel for the (dead-attention) GAT reference.

Effective math (see reference):
    h1  = x @ W1f                 W1f = W1.transpose(1,0,2).reshape(256,128)
    hp1 = elu(adj @ h1)
    h2  = hp1 @ W2f               W2f = W2.transpose(1,0,2).reshape(128,128)
    hp2 = elu(adj @ h2)
    y   = elu(hp2 @ Wout + bout)
    out = log_softmax(y, axis=1)

Distribution: adj is sharded row-wise across 8 cores (2048 rows each),
uploaded pre-transposed + fp16, with the CONTRACTION rows rotated per
core so each core's own nodes come first.  h1 is computed REPLICATED on
every core (x is tiny), so layer 1 needs no collective.  h2 is
exchanged with one fp16 AllGather in feature-major layout; thanks to
the rotation each core starts layer 2 on its own h2 shard (local, no
collective wait) while the AllGather flies, then pulls the other 7
blocks with partition-id-indexed dynamic DMAs and transposes them back
to node-major lhsT tiles on the PE.  Each core streams its 67 MB adj
shard from HBM through the PE array twice:
    hpT[128 feat, 2048 rows] = sum_k h[kblk 128 rows].T-stationary @ adjT[kblk]
fp32 accumulation in PSUM; fp16 on the streamed matmuls (max elementwise
rel err vs the fp32 reference ~6e-4).
"""

import sys

import numpy as np

sys.path.insert(0, "/opt/trn_rl_repo")

N = 16384  # nodes
F = 256  # input features
D = 128  # hidden width (nheads*nhid)
C = 32  # classes
NCORES = 8
S = N // NCORES  # rows per core

_nc_cache = {}


def build_gat_nc(n_total=N, ncores=NCORES, enable_asserts=False, adj_bufs=7, kg=4):
    """Build the SPMD Bass program (one program, runs on all cores)."""
    from concourse import bacc, bass, masks, mybir, tile

    s = n_total // ncores  # shard rows per core
    kb = n_total // 128  # contraction blocks for the big matmul
    kb8 = kb // 8  # x chunk groups
    rc = s // 128  # 128-row chunks in this core's shard
    f32 = mybir.dt.float32
    f16 = mybir.dt.float16
    AF = mybir.ActivationFunctionType
    OP = mybir.AluOpType
    # n-chunks of the big-matmul output (<=512 fp32 per PSUM bank)
    nw = [min(512, s - i) for i in range(0, s, 512)]
    no = [i for i in range(0, s, 512)]

    nc = bacc.Bacc(
        "TRN2",
        target_bir_lowering=False,
        debug=False,
        enable_asserts=enable_asserts,
        num_devices=ncores,
    )

    adjt = nc.dram_tensor("adjt", [n_total, s], f16, kind="ExternalInput")
    xc = nc.dram_tensor("xc", [kb8 * 128, 8 * F], f16, kind="ExternalInput")
    w1 = nc.dram_tensor("w1", [F, D], f16, kind="ExternalInput")
    w2 = nc.dram_tensor("w2", [D, D], f16, kind="ExternalInput")
    wout = nc.dram_tensor("wout", [D, C], f32, kind="ExternalInput")
    bb = nc.dram_tensor("bb", [128, C], f32, kind="ExternalInput")
    # hoff[0, g] = ((rank + 1 + g) % ncores) * 128: gather-block row offsets
    hoff = nc.dram_tensor("hoff", [1, 8], mybir.dt.uint32, kind="ExternalInput")
    out = nc.dram_tensor("out", [s, C], f32, kind="ExternalOutput")

    rg = [list(range(ncores))]

    with tile.TileContext(nc) as tc:
        with (
            tc.tile_pool(name="dram", bufs=1, space="DRAM") as dram,
            tc.tile_pool(name="const", bufs=1) as const,
            tc.tile_pool(name="hfull", bufs=1) as hpool,
            tc.tile_pool(name="adjs", bufs=adj_bufs) as apool,
            tc.tile_pool(name="hblkp", bufs=2) as hblkp,
            tc.tile_pool(name="xcp", bufs=2) as xcpool,
            tc.tile_pool(name="xe", bufs=2) as xepool,
            tc.tile_pool(name="hsb", bufs=2) as hsbpool,
            tc.tile_pool(name="tmp", bufs=1) as tmp,
            tc.tile_pool(name="outp", bufs=2) as outp,
            tc.tile_pool(name="stat", bufs=1) as stat,
            tc.tile_pool(name="psb", bufs=4, space="PSUM") as psb,
            tc.tile_pool(name="pss", bufs=2, space="PSUM") as pss,
            tc.tile_pool(name="psy", bufs=2, space="PSUM") as psy,
        ):
            # two HWDGE rings (sync/scalar) alternate the big adj stream;
            # constants + tiny stores go to the SWDGE path (gpsimd)
            ringA, ringB, ringC = nc.sync, nc.scalar, nc.gpsimd

            # --- replicated constants (SWDGE so rings start streaming) ---
            w1s = const.tile([128, 2, D], f16, tag="w1s")
            ringC.dma_start(w1s[:], w1.ap().rearrange("(a p) m -> p a m", p=128))
            w2s = const.tile([128, D], f16, tag="w2s")
            ringC.dma_start(w2s[:], w2.ap())
            wouts = const.tile([128, C], f32, tag="wouts")
            ringC.dma_start(wouts[:], wout.ap())
            bbs = const.tile([128, C], f32, tag="bbs")
            ringC.dma_start(bbs[:], bb.ap())
            hoffs = const.tile([1, 8], mybir.dt.uint32, tag="hoffs")
            ringC.dma_start(hoffs[:], hoff.ap())
            ident = const.tile([128, 128], f16, tag="ident")
            masks.make_identity(nc, ident[:])

            # --- DRAM bounce buffers for the collective (feature-major) ---
            h2b = dram.tile([128, s], f16, tag="h2b")
            h2f = dram.tile([128 * ncores, s], f16, tag="h2f", addr_space="Shared")

            def big_layer(hs):
                # hpT[128 feat, s rows] += h[kblk].T-stationary @ adjT[kblk]
                ps = [
                    psb.tile([128, w], f32, tag="big", name=f"pbig{i}")
                    for i, w in enumerate(nw)
                ]
                ar = adjt.ap().rearrange("(g j p) m -> g p j m", j=kg, p=128)
                for g in range(kb // kg):
                    at = apool.tile([128, kg, s], f16, tag="adj")
                    (ringA if g % 2 == 0 else ringB).dma_start(at[:], ar[g])
                    for j in range(kg):
                        k = g * kg + j
                        for n, (o, w) in enumerate(zip(no, nw)):
                            nc.tensor.matmul(
                                ps[n][:],
                                hs[:, k, :],
                                at[:, j, o : o + w],
                                start=(k == 0),
                                stop=(k == kb - 1),
                            )
                return ps

            def elu_chunks(ps, dst):
                # dst[:, s] = elu(ps chunks), fp32
                for n, (o, w) in enumerate(zip(no, nw)):
                    neg = tmp.tile([128, 512], f32, tag="neg", name=f"neg{n}")
                    nc.vector.tensor_scalar_min(neg[:, :w], ps[n][:], 0.0)
                    ex = tmp.tile([128, 512], f32, tag="ex", name=f"ex{n}")
                    nc.scalar.activation(ex[:, :w], neg[:, :w], AF.Exp)
                    pm1 = tmp.tile([128, 512], f32, tag="pm1", name=f"pm1{n}")
                    nc.vector.tensor_scalar(
                        pm1[:, :w], ps[n][:], 0.0, -1.0, op0=OP.max, op1=OP.add
                    )
                    nc.vector.tensor_add(dst[:, o : o + w], ex[:, :w], pm1[:, :w])

            # ---- layer 1: h1 replicated (no collective) ----
            # xc group g holds 8 chunk-lhsTs contiguous per partition:
            # xc[g*128+p, ((j*2+a)*128)+m] = xrot.T[a*128+p, (g*8+j)*128+m]
            hs1 = hpool.tile([128, kb, D], f16, tag="hfull")
            xr = xc.ap().rearrange("(g p) q -> g p q", p=128)
            xg = None
            for k in range(kb):
                g, j = divmod(k, 8)
                if j == 0:
                    xg = xcpool.tile([128, 8, 2, 128], f16, tag="xg")
                    (ringA if g % 2 == 0 else ringB).dma_start(
                        xg.rearrange("p j a m -> p (j a m)"), xr[g]
                    )
                ph = pss.tile([128, D], f32, tag="pss", name=f"ph1_{k}")
                nc.tensor.matmul(
                    ph[:], xg[:, j, 0, :], w1s[:, 0, :], start=True, stop=False
                )
                nc.tensor.matmul(
                    ph[:], xg[:, j, 1, :], w1s[:, 1, :], start=False, stop=True
                )
                nc.vector.tensor_copy(hs1[:, k, :], ph[:])
            ps1 = big_layer(hs1)
            x2t = xepool.tile([128, s], f32, tag="xe")
            elu_chunks(ps1, x2t)

            # ---- layer 2 ----
            # own h2 shard (feature-major), start collective, and immediately
            # transpose the local shard into the first rc lhsT chunks
            x2h = xepool.tile([128, s], f16, tag="xeh")
            nc.vector.tensor_copy(x2h[:], x2t[:])
            h2sT = xepool.tile([128, s], f16, tag="h2sT")
            for c in range(rc):
                cs = slice(c * 128, (c + 1) * 128)
                ph2 = pss.tile([128, D], f32, tag="pss", name=f"ph2_{c}")
                # feat-major h2 chunk: W2f.T-stationary @ x2[feat, nodes]
                nc.tensor.matmul(ph2[:], w2s[:], x2h[:, cs], start=True, stop=True)
                nc.vector.tensor_copy(h2sT[:, cs], ph2[:])
            ringC.dma_start(h2b[:], h2sT[:])
            nc.gpsimd.collective_compute(
                "AllGather",
                OP.bypass,
                ins=[h2b.opt()],
                outs=[h2f.opt()],
                replica_groups=rg,
            )
            hs2 = hpool.tile([128, kb, D], f16, tag="hfull")
            for k in range(rc):  # own block: no collective wait
                pt = pss.tile([128, D], f16, tag="pss", name=f"ptl_{k}")
                nc.tensor.transpose(
                    pt[:], h2sT[:, k * 128 : (k + 1) * 128], ident[:]
                )
                nc.vector.tensor_copy(hs2[:, k, :], pt[:])
            # other ranks' blocks: dynamic row offset ((me+1+g) % ncores)*128
            # NOTE: keep these off the sync ring — SP-engine DMAs touching
            # collective-output buffers can hang (test_sync_dma_collective_hang)
            for g in range(ncores - 1):
                with ringB.register(f"hoffr{g}") as hreg:
                    ringB.reg_load(hreg, hoffs[0:1, g : g + 1])
                    off = ringB.snap(hreg, min_val=0, max_val=(ncores - 1) * 128)
                hb = hblkp.tile([128, s], f16, tag="hblk", name=f"hblk{g}")
                ringB.dma_start(hb[:], h2f[bass.ds(off, 128), :])
                for jj in range(rc):
                    k = rc * (1 + g) + jj
                    pt = pss.tile([128, D], f16, tag="pss", name=f"pt_{k}")
                    nc.tensor.transpose(
                        pt[:], hb[:, jj * 128 : (jj + 1) * 128], ident[:]
                    )
                    nc.vector.tensor_copy(hs2[:, k, :], pt[:])
            ps2 = big_layer(hs2)
            x3t = xepool.tile([128, s], f32, tag="xe")
            elu_chunks(ps2, x3t)

            # ---- output layer + log_softmax ----
            outr = out.ap().rearrange("(c p) m -> c p m", p=128)
            zbig = outp.tile([128, rc, C], f32, tag="zbig", bufs=1)
            for c in range(rc):
                py = psy.tile([128, C], f32, tag="psy")
                cs = slice(c * 128, (c + 1) * 128)
                nc.tensor.matmul(py[:], x3t[:, cs], wouts[:], start=True, stop=True)
                nc.vector.tensor_add(zbig[:, c, :], py[:], bbs[:])
            # batched elu over [128, rc*C]
            zf = zbig.rearrange("p c m -> p (c m)")
            negb = tmp.tile([128, rc * C], f32, tag="neg", name="negb")
            nc.vector.tensor_scalar_min(negb[:], zf, 0.0)
            eb = tmp.tile([128, rc * C], f32, tag="ex", name="eb")
            nc.scalar.activation(eb[:], negb[:], AF.Exp)
            pmb = tmp.tile([128, rc * C], f32, tag="pm1", name="pmb")
            nc.vector.tensor_scalar(pmb[:], zf, 0.0, -1.0, op0=OP.max, op1=OP.add)
            zzb = outp.tile([128, rc, C], f32, tag="zzb", bufs=1)
            nc.vector.tensor_add(
                zzb.rearrange("p c m -> p (c m)"), eb[:], pmb[:]
            )
            # batched row-max (negated), then per-chunk exp/lse/final
            negm = stat.tile([128, rc], f32, tag="negm")
            nc.vector.tensor_reduce(
                negm[:], zzb[:], axis=mybir.AxisListType.X, op=OP.max, negate=True
            )
            ssum = stat.tile([128, rc], f32, tag="ssum")
            es = tmp.tile([128, rc * C], f32, tag="neg", name="es")
            esv = es.rearrange("p (c m) -> p c m", m=C)
            for c in range(rc):
                nc.scalar.activation(
                    esv[:, c, :],
                    zzb[:, c, :],
                    AF.Exp,
                    bias=negm[:, c : c + 1],
                    accum_out=ssum[:, c : c + 1],
                )
            lse = stat.tile([128, rc], f32, tag="lse")
            nc.scalar.activation(lse[:], ssum[:], AF.Ln)
            for c in range(rc):
                osb = outp.tile([128, C], f32, tag="osb")
                nc.vector.tensor_scalar(
                    osb[:],
                    zzb[:, c, :],
                    negm[:, c : c + 1],
                    lse[:, c : c + 1],
                    op0=OP.add,
                    op1=OP.subtract,
                )
                ringC.dma_start(outr[c], osb[:])

    nc.compile()
    return nc


def make_in_maps(x, adj, W1, W2, Wout, bout, ncores=NCORES):
    n_total = adj.shape[0]
    s = n_total // ncores
    kb = n_total // 128
    kb8 = kb // 8
    f, d = W1.shape[1], W1.shape[0] * W1.shape[2]
    w1f = np.ascontiguousarray(
        W1.transpose(1, 0, 2).reshape(f, d).astype(np.float16)
    )
    w2f = np.ascontiguousarray(
        W2.transpose(1, 0, 2).reshape(d, d).astype(np.float16)
    )
    woutf = np.ascontiguousarray(Wout.astype(np.float32))
    bbf = np.ascontiguousarray(
        np.broadcast_to(bout.astype(np.float32), (128, Wout.shape[1]))
    )
    adj16 = adj.astype(np.float16)
    x16 = x.astype(np.float16)
    in_maps = []
    for c in range(ncores):
        rows = slice(c * s, (c + 1) * s)
        # rotate contraction rows so this core's own nodes come first
        rot = np.roll(np.arange(n_total), -c * s)
        adjtc = np.ascontiguousarray(adj16[rows][:, rot].T)
        # xc[g*128 + p, ((j*2 + a)*128) + m] = xrot.T[a*128 + p, (g*8 + j)*128 + m]
        xtc = x16[rot].T  # [F, n_total]
        xcf = np.ascontiguousarray(
            xtc.reshape(2, 128, kb8, 8, 128)
            .transpose(2, 1, 3, 0, 4)
            .reshape(kb8 * 128, 8 * f)
        )
        hoffc = np.zeros((1, 8), np.uint32)
        for g in range(ncores - 1):
            hoffc[0, g] = ((c + 1 + g) % ncores) * 128
        in_maps.append(
            {
                "adjt": adjtc,
                "xc": xcf,
                "w1": w1f,
                "w2": w2f,
                "wout": woutf,
                "bb": bbf,
                "hoff": hoffc,
            }
        )
    return in_maps


def kernel(x, adj, W1, W2, Wout, bout):
    from concourse import bass_utils

    x = np.asarray(x)
    adj = np.asarray(adj)
    in_maps = make_in_maps(x, adj, np.asarray(W1), np.asarray(W2),
                           np.asarray(Wout), np.asarray(bout))
    if "nc" not in _nc_cache:
        _nc_cache["nc"] = build_gat_nc()
    res = bass_utils.run_bass_kernel_spmd(
        _nc_cache["nc"], in_maps, core_ids=list(range(NCORES))
    )
    return np.concatenate([r["out"] for r in res.results], axis=0).astype(np.float32)



# revision 6
# speedup vs baseline: 1.2716x; 1.2716x over previous
"""Bass/Trainium2 kernel for the (dead-attention) GAT reference.

Effective math (see reference):
    h1  = x @ W1f                 W1f = W1.transpose(1,0,2).reshape(256,128)
    hp1 = elu(adj @ h1)
    h2  = hp1 @ W2f               W2f = W2.transpose(1,0,2).reshape(128,128)
    hp2 = elu(adj @ h2)
    y   = elu(hp2 @ Wout + bout)
    out = log_softmax(y, axis=1)

v2 design (vs the fp16 baseline):
  * adj is stored as fp8 e3m4 residual  R16 = 16*(adj - 0.5).  Over
    [-8, 8) e3m4 behaves like a uniform 6-bit quantizer (step 1/64 in
    adj units) and the x16 scale keeps all but |adj-0.5|<1/64 out of
    the subnormal range.  This halves the dominant HBM traffic.  The
    rank-1 term 0.5*J@h is restored per feature when reading PSUM:
        hp = elu(psum/16 + 0.5*colsum(h))
    corr1 = 0.5*colsum(x)@W1f is a host-prepared constant; corr2 comes
    from an AllReduce of the per-core colsum(x2) plus one on-device
    fp32 matvec with W2f.
  * adj rows (contraction dim) are laid out in a GLOBAL order (all
    ranks' first-half nodes, then all ranks' second halves), identical
    on every core.  Layer 1 runs as two column-half passes; each pass's
    h2 half is AllGathered (node-major) while the next pass / layer 2
    computes, so the collectives are hidden and layer 2 needs no
    dynamic-offset pulls and no PE transposes: gathered blocks DMA
    straight into the node-major stationary buffer hs2.
  * Layer-1 passes stream their adj groups in REVERSED order so the
    last `adj_bufs` group tiles of both column halves stay resident in
    SBUF and layer 2 re-reads that much less adj from HBM.
  * fp16 stationary (h) x fp8 moving (adj) matmuls, fp32 PSUM.
"""

import sys

import numpy as np

sys.path.insert(0, "/opt/trn_rl_repo")

N = 16384  # nodes
F = 256  # input features
D = 128  # hidden width (nheads*nhid)
C = 32  # classes
NCORES = 8
S = N // NCORES  # rows per core

_nc_cache = {}


def build_gat_nc(n_total=N, ncores=NCORES, enable_asserts=False, adj_bufs=8, kg=4):
    """Build the SPMD Bass program (one program, runs on all cores)."""
    from concourse import bacc, mybir, tile

    s = n_total // ncores  # own rows per core
    half = s // 2  # column-half width
    kb = n_total // 128  # contraction blocks
    ngrp = kb // kg  # adj DMA groups
    kb8 = kb // 8  # xc chunk groups
    c8 = half // 128  # 128-node blocks per (rank, half)
    rc = s // 128  # 128-row chunks of the own shard
    f32 = mybir.dt.float32
    f16 = mybir.dt.float16
    f8 = mybir.dt.float8e3
    AF = mybir.ActivationFunctionType
    OP = mybir.AluOpType

    def chunks(width):
        return [(o, min(512, width - o)) for o in range(0, width, 512)]

    nc = bacc.Bacc(
        "TRN2",
        target_bir_lowering=False,
        debug=False,
        enable_asserts=enable_asserts,
        num_devices=ncores,
    )

    adjt0 = nc.dram_tensor("adjt0", [n_total, half], f8, kind="ExternalInput")
    adjt1 = nc.dram_tensor("adjt1", [n_total, half], f8, kind="ExternalInput")
    xc = nc.dram_tensor("xc", [kb8 * 128, 8 * F], f16, kind="ExternalInput")
    w1 = nc.dram_tensor("w1", [F, D], f16, kind="ExternalInput")
    w2 = nc.dram_tensor("w2", [D, D], f16, kind="ExternalInput")
    wout = nc.dram_tensor("wout", [D, C], f32, kind="ExternalInput")
    bb = nc.dram_tensor("bb", [128, C], f32, kind="ExternalInput")
    corr1 = nc.dram_tensor("corr1", [128, 1], f32, kind="ExternalInput")
    out = nc.dram_tensor("out", [s, C], f32, kind="ExternalOutput")

    rg = [list(range(ncores))]

    with tile.TileContext(nc) as tc:
        with (
            tc.tile_pool(name="dram", bufs=1, space="DRAM") as dram,
            tc.tile_pool(name="const", bufs=1) as const,
            tc.tile_pool(name="hs1p", bufs=1) as hs1p,
            tc.tile_pool(name="hs2p", bufs=1) as hs2p,
            tc.tile_pool(name="ap0", bufs=adj_bufs) as ap0,
            tc.tile_pool(name="ap1", bufs=adj_bufs) as ap1,
            tc.tile_pool(name="xcp", bufs=2) as xcpool,
            tc.tile_pool(name="xe", bufs=2) as xepool,
            tc.tile_pool(name="xh", bufs=2) as xhpool,
            tc.tile_pool(name="h2o", bufs=2) as h2opool,
            tc.tile_pool(name="tmp", bufs=1) as tmp,
            tc.tile_pool(name="outp", bufs=2) as outp,
            tc.tile_pool(name="stat", bufs=1) as stat,
            tc.tile_pool(name="big", bufs=4, space="PSUM") as big,
            tc.tile_pool(name="pss", bufs=2, space="PSUM") as pss,
            tc.tile_pool(name="psy", bufs=2, space="PSUM") as psy,
        ):
            # ring assignment:
            #   A/B (sync + scalar HWDGE): xc and the big adj streams only
            #   V (vector): h2/colsum bounce stores
            #   G (gpsimd): constants, collective triggers, gather pulls,
            #     output stores.  NOTE: keep collective-output reads off the
            #     sync ring (test_sync_dma_collective_hang).
            ringA, ringB = nc.sync, nc.scalar
            ringG = nc.gpsimd

            # --- replicated constants ---
            w1s = const.tile([128, 2, D], f16, tag="w1s")
            ringG.dma_start(w1s[:], w1.ap().rearrange("(a p) m -> p a m", p=128))
            w2s = const.tile([128, D], f16, tag="w2s")
            ringG.dma_start(w2s[:], w2.ap())
            wouts = const.tile([128, C], f32, tag="wouts")
            ringG.dma_start(wouts[:], wout.ap())
            bbs = const.tile([128, C], f32, tag="bbs")
            ringG.dma_start(bbs[:], bb.ap())
            corr1s = const.tile([128, 1], f32, tag="corr1s")
            ringG.dma_start(corr1s[:], corr1.ap())
            # fp32 copy of W2f for the (fp32) corr2 matvec
            w2s32 = const.tile([128, D], f32, tag="w2s32")
            nc.vector.tensor_copy(w2s32[:], w2s[:])

            # --- DRAM bounce buffers for the collectives (node-major) ---
            h2bA = dram.tile([half, D], f16, tag="h2bA")
            h2bB = dram.tile([half, D], f16, tag="h2bB")
            h2fA = dram.tile([ncores * half, D], f16, tag="h2fA", addr_space="Shared")
            h2fB = dram.tile([ncores * half, D], f16, tag="h2fB", addr_space="Shared")
            csb = dram.tile([128, 1], f32, tag="csb")
            csr = dram.tile([128, 1], f32, tag="csr", addr_space="Shared")

            # ---- h1 = x @ W1f, replicated (no collective) ----
            # xc group g holds 8 chunk-lhsTs contiguous per partition:
            # xc[g*128+p, ((j*2+a)*128)+m] = xperm.T[a*128+p, (g*8+j)*128+m]
            hs1 = hs1p.tile([128, kb, D], f16, tag="hs1")
            xr = xc.ap().rearrange("(g p) q -> g p q", p=128)
            xg = None
            for k in range(kb):
                g, j = divmod(k, 8)
                if j == 0:
                    xg = xcpool.tile([128, 8, 2, 128], f16, tag="xg")
                    (ringA if g % 2 == 0 else ringB).dma_start(
                        xg.rearrange("p j a m -> p (j a m)"), xr[g]
                    )
                ph = pss.tile([128, D], f32, tag="pss", name=f"ph1_{k}")
                nc.tensor.matmul(
                    ph[:], xg[:, j, 0, :], w1s[:, 0, :], start=True, stop=False
                )
                nc.tensor.matmul(
                    ph[:], xg[:, j, 1, :], w1s[:, 1, :], start=False, stop=True
                )
                nc.vector.tensor_copy(hs1[:, k, :], ph[:])

            ar0 = adjt0.ap().rearrange("(g p j) m -> g p (j m)", p=128, j=kg)
            ar1 = adjt1.ap().rearrange("(g p j) m -> g p (j m)", p=128, j=kg)

            def stream_pass(ar, apool, nch, inject):
                """One reversed-order streaming pass over adj half `ar`.

                nch: list of (psum_tile, o, w) output chunks.
                inject: {group_iteration_index: callback} runs after that
                group's matmuls are emitted (slots small PE work into the
                stream without stalling its head).
                Returns {g: sbuf tile}; entries g < adj_bufs stay resident.
                """
                tiles = {}
                for gi, g in enumerate(reversed(range(ngrp))):
                    at = apool.tile([128, kg * half], f8, tag="a")
                    ((ringA, ringB)[gi % 2]).dma_start(at[:], ar[g])
                    tiles[g] = at
                    atv = at.rearrange("p (j m) -> p j m", j=kg)
                    for j in range(kg):
                        ki = gi * kg + j
                        k = g * kg + j
                        for ps, o, w in nch:
                            nc.tensor.matmul(
                                ps[:],
                                hs1[:, k, :],
                                atv[:, j, o : o + w],
                                start=(ki == 0),
                                stop=(ki == kb - 1),
                            )
                    if gi in inject:
                        inject[gi]()
                return tiles

            def elu_z(nch, dst, corrap):
                # dst[:, o:o+w] = elu(ps/16 + corr), fp32
                for i, (ps, o, w) in enumerate(nch):
                    z = tmp.tile([128, 512], f32, tag="z", name=f"z{i}")
                    nc.vector.tensor_scalar(
                        z[:, :w], ps[:], 1.0 / 16.0, corrap, op0=OP.mult, op1=OP.add
                    )
                    neg = tmp.tile([128, 512], f32, tag="neg", name=f"neg{i}")
                    nc.vector.tensor_scalar_min(neg[:, :w], z[:, :w], 0.0)
                    ex = tmp.tile([128, 512], f32, tag="ex", name=f"ex{i}")
                    nc.scalar.activation(ex[:, :w], neg[:, :w], AF.Exp)
                    pm1 = tmp.tile([128, 512], f32, tag="pm1", name=f"pm1{i}")
                    nc.vector.tensor_scalar(
                        pm1[:, :w], z[:, :w], 0.0, -1.0, op0=OP.max, op1=OP.add
                    )
                    nc.vector.tensor_add(
                        dst[:, o : o + w], ex[:, :w], pm1[:, :w]
                    )

            x2t = xepool.tile([128, s], f32, tag="xe")
            cs2 = stat.tile([128, 2], f32, tag="cs2")
            hs2 = hs2p.tile([128, kb, D], f16, tag="hs2")
            x2hs = {}

            def half_soft(hx, psn):
                # elu + colsum partial + fp16 cast for column half hx
                def emit():
                    elu_z([(ps, hx * half + o, w) for ps, o, w in psn],
                          x2t, corr1s[:, 0:1])
                    nc.vector.tensor_reduce(
                        cs2[:, hx : hx + 1],
                        x2t[:, hx * half : (hx + 1) * half],
                        axis=mybir.AxisListType.X,
                        op=OP.add,
                    )
                    x2h = xhpool.tile([128, half], f16, tag="xh", name=f"xh{hx}")
                    nc.vector.tensor_copy(x2h[:], x2t[:, hx * half : (hx + 1) * half])
                    x2hs[hx] = x2h

                return emit

            def half_hard(hx, h2b, h2f, slot0):
                # own-h2 (node-major) + bounce + AllGather + static pull
                def emit():
                    x2h = x2hs[hx]
                    h2o = h2opool.tile([128, c8, D], f16, tag="h2o", name=f"h2o{hx}")
                    for c in range(c8):
                        ph = pss.tile([128, D], f32, tag="pss", name=f"h2o_{hx}_{c}")
                        nc.tensor.matmul(
                            ph[:],
                            x2h[:, c * 128 : (c + 1) * 128],
                            w2s[:],
                            start=True,
                            stop=True,
                        )
                        nc.vector.tensor_copy(h2o[:, c, :], ph[:])
                    ringG.dma_start(
                        h2b.rearrange("(c p) m -> p c m", p=128), h2o[:]
                    )
                    ringG.collective_compute(
                        "AllGather",
                        OP.bypass,
                        ins=[h2b.opt()],
                        outs=[h2f.opt()],
                        replica_groups=rg,
                    )
                    ringG.dma_start(
                        hs2[:, slot0 : slot0 + ncores * c8, :],
                        h2f.rearrange("(r c p) m -> p (r c) m", p=128, c=c8),
                    )

                return emit

            # ---- layer 1, pass A (adj columns 0:half) ----
            psA = [
                (big.tile([128, w], f32, tag="big", name=f"pA{i}"), o, w)
                for i, (o, w) in enumerate(chunks(half))
            ]
            tilesA = stream_pass(ar0, ap0, psA, {})

            # ---- layer 1, pass B; half-A epilogue injected into its stream ----
            psB = [
                (big.tile([128, w], f32, tag="big", name=f"pB{i}"), o, w)
                for i, (o, w) in enumerate(chunks(half))
            ]
            hA = min(2, ngrp - 1)
            tilesB = stream_pass(
                ar1, ap1, psB,
                {hA: half_soft(0, psA),
                 min(hA + 1, ngrp - 1): half_hard(0, h2bA, h2fA, 0)},
            )

            # half-B elu must be emitted BEFORE layer 2 reuses psB's banks
            half_soft(1, psB)()

            def cs_epilogue():
                # global colsum(x2): bounce + AllReduce
                csum = stat.tile([128, 1], f32, tag="csum")
                nc.vector.tensor_add(csum[:], cs2[:, 0:1], cs2[:, 1:2])
                ringG.dma_start(csb[:], csum[:])
                ringG.collective_compute(
                    "AllReduce",
                    OP.add,
                    ins=[csb.opt()],
                    outs=[csr.opt()],
                    replica_groups=rg,
                )

            # ---- layer 2: full-width stream, forward order, tile reuse ----
            psL = [
                (big.tile([128, w], f32, tag="big", name=f"pL_{hx}_{i}"), hx, o, w)
                for hx in (0, 1)
                for i, (o, w) in enumerate(chunks(half))
            ]
            hardB = half_hard(1, h2bB, h2fB, ncores * c8)
            for g in range(ngrp):
                if g < adj_bufs:
                    at0, at1 = tilesA[g], tilesB[g]
                else:
                    at0 = ap0.tile([128, kg * half], f8, tag="a")
                    ringA.dma_start(at0[:], ar0[g])
                    at1 = ap1.tile([128, kg * half], f8, tag="a")
                    ringB.dma_start(at1[:], ar1[g])
                a0v = at0.rearrange("p (j m) -> p j m", j=kg)
                a1v = at1.rearrange("p (j m) -> p j m", j=kg)
                for j in range(kg):
                    ki = g * kg + j
                    for ps, hx, o, w in psL:
                        av = a0v if hx == 0 else a1v
                        nc.tensor.matmul(
                            ps[:],
                            hs2[:, ki, :],
                            av[:, j, o : o + w],
                            start=(ki == 0),
                            stop=(ki == kb - 1),
                        )
                if g == min(1, ngrp - 1):
                    hardB()
                    cs_epilogue()

            # corr2 = 0.5 * W2f.T @ colsum(x2)   (fp32 matvec)
            colsum_sb = stat.tile([128, 1], f32, tag="colsum_sb")
            ringG.dma_start(colsum_sb[:], csr[:])
            pcv = psy.tile([128, 1], f32, tag="psy", name="pcv")
            nc.tensor.matmul(pcv[:], w2s32[:], colsum_sb[:], start=True, stop=True)
            corr2s = stat.tile([128, 1], f32, tag="corr2s")
            nc.vector.tensor_scalar_mul(corr2s[:], pcv[:], 0.5)

            x3t = xepool.tile([128, s], f32, tag="xe")
            elu_z(
                [(ps, hx * half + o, w) for ps, hx, o, w in psL],
                x3t,
                corr2s[:, 0:1],
            )

            # ---- output layer + log_softmax ----
            outr = out.ap().rearrange("(c p) m -> c p m", p=128)
            zbig = outp.tile([128, rc, C], f32, tag="zbig", bufs=1)
            for c in range(rc):
                py = psy.tile([128, C], f32, tag="psy")
                cs = slice(c * 128, (c + 1) * 128)
                nc.tensor.matmul(py[:], x3t[:, cs], wouts[:], start=True, stop=True)
                nc.vector.tensor_add(zbig[:, c, :], py[:], bbs[:])
            # batched elu over [128, rc*C]
            zf = zbig.rearrange("p c m -> p (c m)")
            negb = tmp.tile([128, rc * C], f32, tag="neg", name="negb")
            nc.vector.tensor_scalar_min(negb[:], zf, 0.0)
            eb = tmp.tile([128, rc * C], f32, tag="ex", name="eb")
            nc.scalar.activation(eb[:], negb[:], AF.Exp)
            pmb = tmp.tile([128, rc * C], f32, tag="pm1", name="pmb")
            nc.vector.tensor_scalar(pmb[:], zf, 0.0, -1.0, op0=OP.max, op1=OP.add)
            zzb = outp.tile([128, rc, C], f32, tag="zzb", bufs=1)
            nc.vector.tensor_add(zzb.rearrange("p c m -> p (c m)"), eb[:], pmb[:])
            # batched row-max (negated), then per-chunk exp/lse/final
            negm = stat.tile([128, rc], f32, tag="negm")
            nc.vector.tensor_reduce(
                negm[:], zzb[:], axis=mybir.AxisListType.X, op=OP.max, negate=True
            )
            ssum = stat.tile([128, rc], f32, tag="ssum")
            es = tmp.tile([128, rc * C], f32, tag="neg", name="es")
            esv = es.rearrange("p (c m) -> p c m", m=C)
            for c in range(rc):
                nc.scalar.activation(
                    esv[:, c, :],
                    zzb[:, c, :],
                    AF.Exp,
                    bias=negm[:, c : c + 1],
                    accum_out=ssum[:, c : c + 1],
                )
            lse = stat.tile([128, rc], f32, tag="lse")
            nc.scalar.activation(lse[:], ssum[:], AF.Ln)
            for c in range(rc):
                osb = outp.tile([128, C], f32, tag="osb")
                nc.vector.tensor_scalar(
                    osb[:],
                    zzb[:, c, :],
                    negm[:, c : c + 1],
                    lse[:, c : c + 1],
                    op0=OP.add,
                    op1=OP.subtract,
                )
                ringG.dma_start(outr[c], osb[:])

    nc.compile()
    return nc


def make_in_maps(x, adj, W1, W2, Wout, bout, ncores=NCORES, kg=4):
    import ml_dtypes

    n_total = adj.shape[0]
    s = n_total // ncores
    half = s // 2
    kb = n_total // 128
    ngrp = kb // kg
    kb8 = kb // 8
    f, d = W1.shape[1], W1.shape[0] * W1.shape[2]

    # global contraction-row order: all ranks' first halves, then seconds
    perm = np.concatenate(
        [np.arange(r * s, r * s + half) for r in range(ncores)]
        + [np.arange(r * s + half, (r + 1) * s) for r in range(ncores)]
    )

    w1f32 = np.ascontiguousarray(W1.transpose(1, 0, 2).reshape(f, d).astype(np.float32))
    w1f = w1f32.astype(np.float16)
    w2f = np.ascontiguousarray(W2.transpose(1, 0, 2).reshape(d, d).astype(np.float16))
    woutf = np.ascontiguousarray(Wout.astype(np.float32))
    bbf = np.ascontiguousarray(
        np.broadcast_to(bout.astype(np.float32), (128, Wout.shape[1]))
    )
    # corr1 = 0.5 * colsum(h1) = 0.5 * colsum(x) @ W1f
    corr1 = (0.5 * (x.astype(np.float32).sum(0) @ w1f32)).astype(np.float32)
    corr1 = np.ascontiguousarray(corr1.reshape(d, 1))

    # fp8 e3m4 residual of adj, scaled by 16
    r8 = ((adj.astype(np.float32) - 0.5) * 16.0).astype(ml_dtypes.float8_e3m4)

    # xc[g*128 + p, ((j*2 + a)*128) + m] = xperm.T[a*128 + p, (g*8 + j)*128 + m]
    x16 = x.astype(np.float16)
    xtc = x16[perm].T  # [F, n_total]
    xcf = np.ascontiguousarray(
        xtc.reshape(2, 128, kb8, 8, 128)
        .transpose(2, 1, 3, 0, 4)
        .reshape(kb8 * 128, 8 * f)
    )

    def reorder(t):
        # dram row (g*kg*128 + p*kg + j) <- k-row (g*kg*128 + j*128 + p)
        return np.ascontiguousarray(
            t.reshape(ngrp, kg, 128, t.shape[1]).transpose(0, 2, 1, 3).reshape(t.shape)
        )

    in_maps = []
    for c in range(ncores):
        t0 = r8[c * s : c * s + half, :][:, perm].T  # [n_total, half]
        t1 = r8[c * s + half : (c + 1) * s, :][:, perm].T
        in_maps.append(
            {
                "adjt0": reorder(t0),
                "adjt1": reorder(t1),
                "xc": xcf,
                "w1": w1f,
                "w2": w2f,
                "wout": woutf,
                "bb": bbf,
                "corr1": corr1,
            }
        )
    return in_maps


def kernel(x, adj, W1, W2, Wout, bout):
    from concourse import bass_utils

    x = np.asarray(x)
    adj = np.asarray(adj)
    in_maps = make_in_maps(x, adj, np.asarray(W1), np.asarray(W2),
                           np.asarray(Wout), np.asarray(bout))
    if "nc" not in _nc_cache:
        _nc_cache["nc"] = build_gat_nc()
    res = bass_utils.run_bass_kernel_spmd(
        _nc_cache["nc"], in_maps, core_ids=list(range(NCORES))
    )
    return np.concatenate([r["out"] for r in res.results], axis=0).astype(np.float32)


# revision 9
# speedup vs baseline: 1.5535x; 1.2217x over previous
"""Bass/Trainium2 kernel for the (dead-attention) GAT reference.

Effective math (see reference):
    h1  = x @ W1f                 W1f = W1.transpose(1,0,2).reshape(256,128)
    hp1 = elu(adj @ h1)
    h2  = hp1 @ W2f               W2f = W2.transpose(1,0,2).reshape(128,128)
    hp2 = elu(adj @ h2)
    y   = elu(hp2 @ Wout + bout)
    out = log_softmax(y, axis=1)

v3 design:
  * adj stored as fp8 e4m3 residual  R16 = 16*(adj - 0.5)  and h1/h2
    stationaries in e4m3, so every big matmul runs in DoubleRow perf
    mode: one instruction contracts a PAIR of 128-row k-blocks at 0.5
    cycles/row.  This halves HBM traffic (vs fp16) and roughly halves
    PE instruction count/stream time.  The rank-1 term 0.5*J@h lost by
    the residual encoding is restored per feature when reading PSUM:
        hp = elu(psum/16 + 0.5*colsum(h))
    corr1 = 0.5*colsum(x)@W1f is a host-prepared constant; corr2 is
    assembled from per-core colsum(x2) partials carried in the gather
    payloads as e4m3 hi/lo pairs, then one on-device fp32 matvec.
    Verified numerics vs fp32 reference (host emulation): 3.1e-4.
  * adj rows (contraction dim) use a GLOBAL order (all ranks' first-
    half nodes, then all ranks' second halves), identical on every
    core.  Layer 1 runs as two column-half passes; each pass's h2 half
    is AllGathered while subsequent compute streams, hiding the
    collectives.  Payloads are partition-major (one DMA line per
    partition) and gathered blocks DMA straight into the node-major
    stationary buffer hs2 with one plain 2D copy per rank - no PE
    transposes, no dynamic offsets.
  * Layer-1 passes stream their adj groups in REVERSED order so the
    last `adj_bufs` group tiles of both column halves stay resident in
    SBUF and layer 2 re-reads that much less adj from HBM.
"""

import sys

import numpy as np

sys.path.insert(0, "/opt/trn_rl_repo")

N = 16384  # nodes
F = 256  # input features
D = 128  # hidden width (nheads*nhid)
C = 32  # classes
NCORES = 8
S = N // NCORES  # rows per core

_nc_cache = {}


def build_gat_nc(n_total=N, ncores=NCORES, enable_asserts=False, adj_bufs=10, kg=4):
    """Build the SPMD Bass program (one program, runs on all cores)."""
    from concourse import bacc, mybir, tile

    s = n_total // ncores  # own rows per core
    half = s // 2  # column-half width
    kb = n_total // 128  # contraction blocks
    ngrp = kb // kg  # adj DMA groups
    kb8 = kb // 8  # xc chunk groups
    c8 = half // 128  # 128-node blocks per (rank, half)
    rc = s // 128  # 128-row chunks of the own shard
    f32 = mybir.dt.float32
    f16 = mybir.dt.float16
    f8 = mybir.dt.float8e4
    AF = mybir.ActivationFunctionType
    OP = mybir.AluOpType
    DR = mybir.MatmulPerfMode.DoubleRow

    def chunks(width):
        return [(o, min(512, width - o)) for o in range(0, width, 512)]

    nc = bacc.Bacc(
        "TRN2",
        target_bir_lowering=False,
        debug=False,
        enable_asserts=enable_asserts,
        num_devices=ncores,
    )

    adjt0 = nc.dram_tensor("adjt0", [n_total, half], f8, kind="ExternalInput")
    adjt1 = nc.dram_tensor("adjt1", [n_total, half], f8, kind="ExternalInput")
    xc = nc.dram_tensor("xc", [kb8 * 128, 8 * F], f8, kind="ExternalInput")
    w1 = nc.dram_tensor("w1", [F, D], f8, kind="ExternalInput")
    w2 = nc.dram_tensor("w2", [D, D], f16, kind="ExternalInput")
    wout = nc.dram_tensor("wout", [D, C], f32, kind="ExternalInput")
    bb = nc.dram_tensor("bb", [128, C], f32, kind="ExternalInput")
    corr1 = nc.dram_tensor("corr1", [128, 1], f32, kind="ExternalInput")
    out = nc.dram_tensor("out", [s, C], f32, kind="ExternalOutput")

    rg = [list(range(ncores))]
    pw = c8 * 128 + 2  # payload width: c8 h2 blocks + colsum hi/lo

    with tile.TileContext(nc) as tc:
        with (
            tc.tile_pool(name="dram", bufs=1, space="DRAM") as dram,
            tc.tile_pool(name="const", bufs=1) as const,
            tc.tile_pool(name="hs1p", bufs=1) as hs1p,
            tc.tile_pool(name="hs2p", bufs=1) as hs2p,
            tc.tile_pool(name="ap0", bufs=adj_bufs) as ap0,
            tc.tile_pool(name="ap1", bufs=adj_bufs) as ap1,
            tc.tile_pool(name="xcp", bufs=2) as xcpool,
            tc.tile_pool(name="xe", bufs=2) as xepool,
            tc.tile_pool(name="xh", bufs=2) as xhpool,
            tc.tile_pool(name="h2o", bufs=2) as h2opool,
            tc.tile_pool(name="tmp", bufs=1) as tmp,
            tc.tile_pool(name="outp", bufs=2) as outp,
            tc.tile_pool(name="stat", bufs=1) as stat,
            tc.tile_pool(name="big", bufs=4, space="PSUM") as big,
            tc.tile_pool(name="pss", bufs=2, space="PSUM") as pss,
            tc.tile_pool(name="psy", bufs=2, space="PSUM") as psy,
        ):
            # ring assignment:
            #   A/B (sync + scalar HWDGE): xc and the big adj streams only
            #   G (gpsimd SWDGE): constants, bounces, collective triggers,
            #     gather pulls, output stores.  NOTE: keep collective-output
            #     reads off the sync ring (test_sync_dma_collective_hang).
            ringA, ringB = nc.sync, nc.scalar
            ringG = nc.gpsimd

            # --- replicated constants ---
            w1s = const.tile([128, 2, D], f8, tag="w1s")
            ringG.dma_start(w1s[:], w1.ap().rearrange("(a p) m -> p a m", p=128))
            w2s = const.tile([128, D], f16, tag="w2s")
            ringG.dma_start(w2s[:], w2.ap())
            wouts = const.tile([128, C], f32, tag="wouts")
            ringG.dma_start(wouts[:], wout.ap())
            bbs = const.tile([128, C], f32, tag="bbs")
            ringG.dma_start(bbs[:], bb.ap())
            corr1s = const.tile([128, 1], f32, tag="corr1s")
            ringG.dma_start(corr1s[:], corr1.ap())
            # fp32 copy of W2f for the (fp32) corr2 matvec
            w2s32 = const.tile([128, D], f32, tag="w2s32")
            nc.vector.tensor_copy(w2s32[:], w2s[:])

            # --- DRAM bounce buffers for the collectives (partition-major) ---
            h2bA = dram.tile([128, pw], f8, tag="h2bA")
            h2bB = dram.tile([128, pw], f8, tag="h2bB")
            h2fA = dram.tile([ncores * 128, pw], f8, tag="h2fA", addr_space="Shared")
            h2fB = dram.tile([ncores * 128, pw], f8, tag="h2fB", addr_space="Shared")

            # ---- h1 = x @ W1f, replicated (DoubleRow over the two F-halves) ----
            # xc group g holds 8 chunk-lhsTs contiguous per partition:
            # xc[g*128+p, ((j*2+a)*128)+m] = xperm.T[a*128+p, (g*8+j)*128+m]
            hs1 = hs1p.tile([128, kb, D], f8, tag="hs1")
            xr = xc.ap().rearrange("(g p) q -> g p q", p=128)
            xg = None
            for k in range(kb):
                g, j = divmod(k, 8)
                if j == 0:
                    xg = xcpool.tile([128, 8, 2, 128], f8, tag="xg")
                    (ringA if g % 2 == 0 else ringB).dma_start(
                        xg.rearrange("p j a m -> p (j a m)"), xr[g]
                    )
                ph = pss.tile([128, D], f32, tag="pss", name=f"ph1_{k}")
                nc.tensor.matmul(
                    ph[:], xg[:, j, :, :], w1s[:], start=True, stop=True,
                    perf_mode=DR,
                )
                nc.vector.tensor_copy(hs1[:, k, :], ph[:])

            ar0 = adjt0.ap().rearrange("(g p j) m -> g p (j m)", p=128, j=kg)
            ar1 = adjt1.ap().rearrange("(g p j) m -> g p (j m)", p=128, j=kg)

            def stream_pass(ar, apool, nch, inject):
                """One reversed-order DoubleRow streaming pass over `ar`.

                nch: list of (psum_tile, o, w) output chunks.
                inject: {group_iteration_index: callback} runs after that
                group's matmuls are emitted.
                Returns {g: sbuf tile}; entries g < adj_bufs stay resident.
                """
                tiles = {}
                kgp = kg // 2
                for gi, g in enumerate(reversed(range(ngrp))):
                    at = apool.tile([128, kg * half], f8, tag="a")
                    ((ringA, ringB)[gi % 2]).dma_start(at[:], ar[g])
                    tiles[g] = at
                    atv = at.rearrange("p (j m) -> p j m", j=kg)
                    for jp in range(kgp):
                        kpi = gi * kgp + jp
                        kp = g * kg + 2 * jp  # first k-block of the pair
                        for ps, o, w in nch:
                            nc.tensor.matmul(
                                ps[:],
                                hs1[:, kp : kp + 2, :],
                                atv[:, 2 * jp : 2 * jp + 2, o : o + w],
                                start=(kpi == 0),
                                stop=(kpi == kb // 2 - 1),
                                perf_mode=DR,
                            )
                    if gi in inject:
                        inject[gi]()
                return tiles

            def elu_z(ps, o, w, dst, corrap, i):
                # dst[:, o:o+w] = elu(ps/16 + corr), fp32
                z = tmp.tile([128, 512], f32, tag="z", name=f"z{i}")
                nc.vector.tensor_scalar(
                    z[:, :w], ps[:], 1.0 / 16.0, corrap, op0=OP.mult, op1=OP.add
                )
                neg = tmp.tile([128, 512], f32, tag="neg", name=f"neg{i}")
                nc.vector.tensor_scalar_min(neg[:, :w], z[:, :w], 0.0)
                ex = tmp.tile([128, 512], f32, tag="ex", name=f"ex{i}")
                nc.scalar.activation(ex[:, :w], neg[:, :w], AF.Exp)
                pm1 = tmp.tile([128, 512], f32, tag="pm1", name=f"pm1{i}")
                nc.vector.tensor_scalar(
                    pm1[:, :w], z[:, :w], 0.0, -1.0, op0=OP.max, op1=OP.add
                )
                nc.vector.tensor_add(dst[:, o : o + w], ex[:, :w], pm1[:, :w])

            x2t = xepool.tile([128, s], f32, tag="xe")
            cs2 = stat.tile([128, 2], f32, tag="cs2")
            hs2 = hs2p.tile([128, kb, D], f8, tag="hs2")
            x2hs = {}
            cshl = {}

            def half_soft(hx, psn):
                # elu + colsum partial (hi/lo e4m3) + fp16 cast, half hx
                def emit():
                    for i, (ps, o, w) in enumerate(psn):
                        elu_z(ps, hx * half + o, w, x2t, corr1s[:, 0:1], i)
                    nc.vector.tensor_reduce(
                        cs2[:, hx : hx + 1],
                        x2t[:, hx * half : (hx + 1) * half],
                        axis=mybir.AxisListType.X,
                        op=OP.add,
                    )
                    # hi/lo e4m3 encoding of colsum/2048 for the payload
                    t = stat.tile([128, 1], f32, tag=f"cst{hx}")
                    nc.vector.tensor_scalar_mul(t[:], cs2[:, hx : hx + 1], 1.0 / 2048.0)
                    hi8 = stat.tile([128, 1], f8, tag=f"hi8{hx}")
                    nc.vector.tensor_copy(hi8[:], t[:])
                    hi32 = stat.tile([128, 1], f32, tag=f"hi32{hx}")
                    nc.vector.tensor_copy(hi32[:], hi8[:])
                    lo = stat.tile([128, 1], f32, tag=f"lo{hx}")
                    nc.vector.tensor_sub(lo[:], t[:], hi32[:])
                    lo8 = stat.tile([128, 1], f8, tag=f"lo8{hx}")
                    nc.vector.tensor_scalar_mul(lo8[:], lo[:], 16.0)
                    cshl[hx] = (hi8, lo8)
                    x2h = xhpool.tile([128, half], f16, tag="xh", name=f"xh{hx}")
                    nc.vector.tensor_copy(x2h[:], x2t[:, hx * half : (hx + 1) * half])
                    x2hs[hx] = x2h

                return emit

            def half_hard(hx, h2b, h2f, slot0):
                # own-h2 (node-major) + bounce + AllGather + per-rank pulls
                def emit():
                    x2h = x2hs[hx]
                    h2o = h2opool.tile([128, c8, D], f8, tag="h2o", name=f"h2o{hx}")
                    for c in range(c8):
                        ph = pss.tile([128, D], f32, tag="pss", name=f"h2o_{hx}_{c}")
                        nc.tensor.matmul(
                            ph[:],
                            x2h[:, c * 128 : (c + 1) * 128],
                            w2s[:],
                            start=True,
                            stop=True,
                        )
                        nc.vector.tensor_copy(h2o[:, c, :], ph[:])
                    ringG.dma_start(h2b[:, 0 : c8 * 128], h2o[:])
                    hi8, lo8 = cshl[hx]
                    ringG.dma_start(h2b[:, c8 * 128 : c8 * 128 + 1], hi8[:])
                    ringG.dma_start(h2b[:, c8 * 128 + 1 : c8 * 128 + 2], lo8[:])
                    ringG.collective_compute(
                        "AllGather",
                        OP.bypass,
                        ins=[h2b.opt()],
                        outs=[h2f.opt()],
                        replica_groups=rg,
                    )
                    for r in range(ncores):
                        ringG.dma_start(
                            hs2[:, slot0 + r * c8 : slot0 + (r + 1) * c8, :],
                            h2f[r * 128 : (r + 1) * 128, 0 : c8 * 128],
                        )

                return emit

            # ---- layer 1, pass A (adj columns 0:half) ----
            psA = [
                (big.tile([128, w], f32, tag="big", name=f"pA{i}"), o, w)
                for i, (o, w) in enumerate(chunks(half))
            ]
            tilesA = stream_pass(ar0, ap0, psA, {})

            # ---- layer 1, pass B; half-A epilogue injected into its stream ----
            psB = [
                (big.tile([128, w], f32, tag="big", name=f"pB{i}"), o, w)
                for i, (o, w) in enumerate(chunks(half))
            ]
            hA = min(2, ngrp - 1)
            tilesB = stream_pass(
                ar1, ap1, psB,
                {hA: half_soft(0, psA),
                 min(hA + 1, ngrp - 1): half_hard(0, h2bA, h2fA, 0)},
            )

            # half-B elu must be emitted BEFORE layer 2 reuses psB's banks
            half_soft(1, psB)()

            # ---- layer 2: full-width DoubleRow stream, forward order ----
            psL = [
                (big.tile([128, w], f32, tag="big", name=f"pL_{hx}_{i}"), hx, o, w)
                for hx in (0, 1)
                for i, (o, w) in enumerate(chunks(half))
            ]
            hardB = half_hard(1, h2bB, h2fB, ncores * c8)
            kgp = kg // 2
            for g in range(ngrp):
                if g < adj_bufs:
                    at0, at1 = tilesA[g], tilesB[g]
                else:
                    at0 = ap0.tile([128, kg * half], f8, tag="a")
                    ringA.dma_start(at0[:], ar0[g])
                    at1 = ap1.tile([128, kg * half], f8, tag="a")
                    ringB.dma_start(at1[:], ar1[g])
                a0v = at0.rearrange("p (j m) -> p j m", j=kg)
                a1v = at1.rearrange("p (j m) -> p j m", j=kg)
                for jp in range(kgp):
                    kpi = g * kgp + jp
                    kp = g * kg + 2 * jp
                    for ps, hx, o, w in psL:
                        av = a0v if hx == 0 else a1v
                        nc.tensor.matmul(
                            ps[:],
                            hs2[:, kp : kp + 2, :],
                            av[:, 2 * jp : 2 * jp + 2, o : o + w],
                            start=(kpi == 0),
                            stop=(kpi == kb // 2 - 1),
                            perf_mode=DR,
                        )
                if g == min(1, ngrp - 1):
                    hardB()

            # corr2 = 0.5 * W2f.T @ colsum(x2): assemble from gathered hi/lo
            parts = stat.tile([128, 2, 2 * ncores], f8, tag="parts")
            for hx, h2f in ((0, h2fA), (1, h2fB)):
                hv = h2f.rearrange("(r p) q -> q p r", p=128)
                for t in (0, 1):
                    ringG.dma_start(
                        parts[:, t, hx * ncores : (hx + 1) * ncores],
                        hv[c8 * 128 + t],
                    )
            partsf = stat.tile([128, 2, 2 * ncores], f32, tag="partsf")
            nc.vector.tensor_copy(partsf[:], parts[:])
            ch = stat.tile([128, 2], f32, tag="ch")
            nc.vector.tensor_reduce(
                ch[:, 0:1], partsf[:, 0, :], axis=mybir.AxisListType.X, op=OP.add
            )
            nc.vector.tensor_reduce(
                ch[:, 1:2], partsf[:, 1, :], axis=mybir.AxisListType.X, op=OP.add
            )
            # colsum(x2) = 2048*CH + 128*CL = 128*(16*CH + CL)
            csum = stat.tile([128, 1], f32, tag="csum")
            nc.vector.tensor_scalar(
                csum[:], ch[:, 0:1], 16.0, ch[:, 1:2], op0=OP.mult, op1=OP.add
            )
            pcv = psy.tile([128, 1], f32, tag="psy", name="pcv")
            nc.tensor.matmul(pcv[:], w2s32[:], csum[:], start=True, stop=True)
            # corr2 = 0.5 * 128 * pcv
            corr2s = stat.tile([128, 1], f32, tag="corr2s")
            nc.vector.tensor_scalar_mul(corr2s[:], pcv[:], 64.0)

            # ---- pipelined tail: per-chunk elu -> out matmuls; then softmax ----
            x3t = xepool.tile([128, s], f32, tag="xe")
            outr = out.ap().rearrange("(c p) m -> c p m", p=128)
            zbig = outp.tile([128, rc, C], f32, tag="zbig", bufs=1)
            for i, (ps, hx, o, w) in enumerate(psL):
                off = hx * half + o
                elu_z(ps, off, w, x3t, corr2s[:, 0:1], i)
                for c in range(off // 128, (off + w) // 128):
                    py = psy.tile([128, C], f32, tag="psy")
                    cs = slice(c * 128, (c + 1) * 128)
                    nc.tensor.matmul(
                        py[:], x3t[:, cs], wouts[:], start=True, stop=True
                    )
                    nc.vector.tensor_add(zbig[:, c, :], py[:], bbs[:])
            # batched elu over [128, rc*C]
            zf = zbig.rearrange("p c m -> p (c m)")
            negb = tmp.tile([128, rc * C], f32, tag="neg", name="negb")
            nc.vector.tensor_scalar_min(negb[:], zf, 0.0)
            eb = tmp.tile([128, rc * C], f32, tag="ex", name="eb")
            nc.scalar.activation(eb[:], negb[:], AF.Exp)
            pmb = tmp.tile([128, rc * C], f32, tag="pm1", name="pmb")
            nc.vector.tensor_scalar(pmb[:], zf, 0.0, -1.0, op0=OP.max, op1=OP.add)
            zzb = outp.tile([128, rc, C], f32, tag="zzb", bufs=1)
            nc.vector.tensor_add(zzb.rearrange("p c m -> p (c m)"), eb[:], pmb[:])
            # batched row-max (negated), then per-chunk exp/lse/final
            negm = stat.tile([128, rc], f32, tag="negm")
            nc.vector.tensor_reduce(
                negm[:], zzb[:], axis=mybir.AxisListType.X, op=OP.max, negate=True
            )
            ssum = stat.tile([128, rc], f32, tag="ssum")
            es = tmp.tile([128, rc * C], f32, tag="neg", name="es")
            esv = es.rearrange("p (c m) -> p c m", m=C)
            for c in range(rc):
                nc.scalar.activation(
                    esv[:, c, :],
                    zzb[:, c, :],
                    AF.Exp,
                    bias=negm[:, c : c + 1],
                    accum_out=ssum[:, c : c + 1],
                )
            lse = stat.tile([128, rc], f32, tag="lse")
            nc.scalar.activation(lse[:], ssum[:], AF.Ln)
            for c in range(rc):
                osb = outp.tile([128, C], f32, tag="osb")
                nc.vector.tensor_scalar(
                    osb[:],
                    zzb[:, c, :],
                    negm[:, c : c + 1],
                    lse[:, c : c + 1],
                    op0=OP.add,
                    op1=OP.subtract,
                )
                ringG.dma_start(outr[c], osb[:])

    nc.compile()
    return nc


def make_in_maps(x, adj, W1, W2, Wout, bout, ncores=NCORES, kg=4):
    import ml_dtypes

    f8np = ml_dtypes.float8_e4m3
    n_total = adj.shape[0]
    s = n_total // ncores
    half = s // 2
    kb = n_total // 128
    ngrp = kb // kg
    kb8 = kb // 8
    f, d = W1.shape[1], W1.shape[0] * W1.shape[2]

    # global contraction-row order: all ranks' first halves, then seconds
    perm = np.concatenate(
        [np.arange(r * s, r * s + half) for r in range(ncores)]
        + [np.arange(r * s + half, (r + 1) * s) for r in range(ncores)]
    )

    w1f32 = np.ascontiguousarray(W1.transpose(1, 0, 2).reshape(f, d).astype(np.float32))
    w1f = w1f32.astype(f8np)
    w2f = np.ascontiguousarray(W2.transpose(1, 0, 2).reshape(d, d).astype(np.float16))
    woutf = np.ascontiguousarray(Wout.astype(np.float32))
    bbf = np.ascontiguousarray(
        np.broadcast_to(bout.astype(np.float32), (128, Wout.shape[1]))
    )
    # corr1 = 0.5 * colsum(h1) = 0.5 * colsum(x) @ W1f
    corr1 = (0.5 * (x.astype(np.float32).sum(0) @ w1f32)).astype(np.float32)
    corr1 = np.ascontiguousarray(corr1.reshape(d, 1))

    # fp8 e4m3 residual of adj, scaled by 16
    r8 = ((adj.astype(np.float32) - 0.5) * 16.0).astype(f8np)

    # xc[g*128 + p, ((j*2 + a)*128) + m] = xperm.T[a*128 + p, (g*8 + j)*128 + m]
    x8 = x.astype(f8np)
    xtc = x8[perm].T  # [F, n_total]
    xcf = np.ascontiguousarray(
        xtc.reshape(2, 128, kb8, 8, 128)
        .transpose(2, 1, 3, 0, 4)
        .reshape(kb8 * 128, 8 * f)
    )

    def reorder(t):
        # dram row (g*kg*128 + p*kg + j) <- k-row (g*kg*128 + j*128 + p)
        return np.ascontiguousarray(
            t.reshape(ngrp, kg, 128, t.shape[1]).transpose(0, 2, 1, 3).reshape(t.shape)
        )

    in_maps = []
    for c in range(ncores):
        t0 = r8[c * s : c * s + half, :][:, perm].T  # [n_total, half]
        t1 = r8[c * s + half : (c + 1) * s, :][:, perm].T
        in_maps.append(
            {
                "adjt0": reorder(t0),
                "adjt1": reorder(t1),
                "xc": xcf,
                "w1": w1f,
                "w2": w2f,
                "wout": woutf,
                "bb": bbf,
                "corr1": corr1,
            }
        )
    return in_maps


def kernel(x, adj, W1, W2, Wout, bout):
    from concourse import bass_utils

    x = np.asarray(x)
    adj = np.asarray(adj)
    in_maps = make_in_maps(x, adj, np.asarray(W1), np.asarray(W2),
                           np.asarray(Wout), np.asarray(bout))
    if "nc" not in _nc_cache:
        _nc_cache["nc"] = build_gat_nc()
    res = bass_utils.run_bass_kernel_spmd(
        _nc_cache["nc"], in_maps, core_ids=list(range(NCORES))
    )
    return np.concatenate([r["out"] for r in res.results], axis=0).astype(np.float32)


# revision 11
# speedup vs baseline: 1.7607x; 1.1334x over previous
"""Bass/Trainium2 kernel for the (dead-attention) GAT reference.

Effective math (see reference):
    h1  = x @ W1f                 W1f = W1.transpose(1,0,2).reshape(256,128)
    hp1 = elu(adj @ h1)
    h2  = hp1 @ W2f               W2f = W2.transpose(1,0,2).reshape(128,128)
    hp2 = elu(adj @ h2)
    y   = elu(hp2 @ Wout + bout)
    out = log_softmax(y, axis=1)

v3 design:
  * adj stored as fp8 e4m3 residual  R16 = 16*(adj - 0.5)  and h1/h2
    stationaries in e4m3, so every big matmul runs in DoubleRow perf
    mode: one instruction contracts a PAIR of 128-row k-blocks at 0.5
    cycles/row.  This halves HBM traffic (vs fp16) and roughly halves
    PE instruction count/stream time.  The rank-1 term 0.5*J@h lost by
    the residual encoding is restored per feature when reading PSUM:
        hp = elu(psum/16 + 0.5*colsum(h))
    corr1 = 0.5*colsum(x)@W1f is a host-prepared constant; corr2 is
    assembled from per-core colsum(x2) partials carried in the gather
    payloads as e4m3 hi/lo pairs, then one on-device fp32 matvec.
    Verified numerics vs fp32 reference (host emulation): 3.1e-4.
  * adj rows (contraction dim) use a GLOBAL order (all ranks' first-
    half nodes, then all ranks' second halves), identical on every
    core.  Layer 1 runs as two column-half passes; each pass's h2 half
    is AllGathered while subsequent compute streams, hiding the
    collectives.  Payloads are partition-major (one DMA line per
    partition) and gathered blocks DMA straight into the node-major
    stationary buffer hs2 with one plain 2D copy per rank - no PE
    transposes, no dynamic offsets.
  * Layer-1 passes stream their adj groups in REVERSED order so the
    last `adj_bufs` group tiles of both column halves stay resident in
    SBUF and layer 2 re-reads that much less adj from HBM.
"""

import sys

import numpy as np

sys.path.insert(0, "/opt/trn_rl_repo")

N = 16384  # nodes
F = 256  # input features
D = 128  # hidden width (nheads*nhid)
C = 32  # classes
NCORES = 8
S = N // NCORES  # rows per core

_nc_cache = {}


def build_gat_nc(n_total=N, ncores=NCORES, enable_asserts=False, adj_bufs=12, kg=4):
    """Build the SPMD Bass program (one program, runs on all cores)."""
    from concourse import bacc, mybir, tile

    s = n_total // ncores  # own rows per core
    half = s // 2  # column-half width
    kb = n_total // 128  # contraction blocks
    ngrp = kb // kg  # adj DMA groups
    kb8 = kb // 8  # xc chunk groups
    c8 = half // 128  # 128-node blocks per (rank, half)
    rc = s // 128  # 128-row chunks of the own shard
    f32 = mybir.dt.float32
    f16 = mybir.dt.float16
    f8 = mybir.dt.float8e4
    AF = mybir.ActivationFunctionType
    OP = mybir.AluOpType
    DR = mybir.MatmulPerfMode.DoubleRow

    def chunks(width):
        return [(o, min(512, width - o)) for o in range(0, width, 512)]

    nc = bacc.Bacc(
        "TRN2",
        target_bir_lowering=False,
        debug=False,
        enable_asserts=enable_asserts,
        num_devices=ncores,
    )

    adjt0 = nc.dram_tensor("adjt0", [n_total, half], f8, kind="ExternalInput")
    adjt1 = nc.dram_tensor("adjt1", [n_total, half], f8, kind="ExternalInput")
    xc = nc.dram_tensor("xc", [kb8 * 128, 8 * F], f8, kind="ExternalInput")
    w1 = nc.dram_tensor("w1", [F, D], f8, kind="ExternalInput")
    w2 = nc.dram_tensor("w2", [D, D], f16, kind="ExternalInput")
    wout = nc.dram_tensor("wout", [D, C], f32, kind="ExternalInput")
    bb = nc.dram_tensor("bb", [128, C], f32, kind="ExternalInput")
    corr1 = nc.dram_tensor("corr1", [128, 1], f32, kind="ExternalInput")
    out = nc.dram_tensor("out", [s, C], f32, kind="ExternalOutput")

    rg = [list(range(ncores))]
    pw = c8 * 128 + 2  # payload width: c8 h2 blocks + colsum hi/lo

    with tile.TileContext(nc) as tc:
        with (
            tc.tile_pool(name="dram", bufs=1, space="DRAM") as dram,
            tc.tile_pool(name="const", bufs=1) as const,
            tc.tile_pool(name="hs1p", bufs=1) as hs1p,
            tc.tile_pool(name="hs2p", bufs=1) as hs2p,
            tc.tile_pool(name="ap0", bufs=adj_bufs) as ap0,
            tc.tile_pool(name="ap1", bufs=adj_bufs) as ap1,
            tc.tile_pool(name="xcp", bufs=2) as xcpool,
            tc.tile_pool(name="xe", bufs=2) as xepool,
            tc.tile_pool(name="xh", bufs=2) as xhpool,
            tc.tile_pool(name="h2o", bufs=2) as h2opool,
            tc.tile_pool(name="tmp", bufs=1) as tmp,
            tc.tile_pool(name="outp", bufs=2) as outp,
            tc.tile_pool(name="stat", bufs=1) as stat,
            tc.tile_pool(name="big", bufs=4, space="PSUM") as big,
            tc.tile_pool(name="pss", bufs=2, space="PSUM") as pss,
            tc.tile_pool(name="psy", bufs=2, space="PSUM") as psy,
        ):
            # ring assignment:
            #   A/B (sync + scalar HWDGE): xc and the big adj streams only
            #   G (gpsimd SWDGE): constants, bounces, collective triggers,
            #     gather pulls, output stores.  NOTE: keep collective-output
            #     reads off the sync ring (test_sync_dma_collective_hang).
            ringA, ringB = nc.sync, nc.scalar
            ringG = nc.gpsimd

            # --- replicated constants ---
            w1s = const.tile([128, 2, D], f8, tag="w1s")
            ringG.dma_start(w1s[:], w1.ap().rearrange("(a p) m -> p a m", p=128))
            w2s = const.tile([128, D], f16, tag="w2s")
            ringG.dma_start(w2s[:], w2.ap())
            wouts = const.tile([128, C], f32, tag="wouts")
            ringG.dma_start(wouts[:], wout.ap())
            bbs = const.tile([128, C], f32, tag="bbs")
            ringG.dma_start(bbs[:], bb.ap())
            corr1s = const.tile([128, 1], f32, tag="corr1s")
            ringG.dma_start(corr1s[:], corr1.ap())
            # fp32 copy of W2f for the (fp32) corr2 matvec
            w2s32 = const.tile([128, D], f32, tag="w2s32")
            nc.vector.tensor_copy(w2s32[:], w2s[:])

            # --- DRAM bounce buffers for the collectives (partition-major) ---
            h2bA = dram.tile([128, pw], f8, tag="h2bA")
            h2bB = dram.tile([128, pw], f8, tag="h2bB")
            h2fA = dram.tile([ncores * 128, pw], f8, tag="h2fA", addr_space="Shared")
            h2fB = dram.tile([ncores * 128, pw], f8, tag="h2fB", addr_space="Shared")

            # ---- h1 = x @ W1f, replicated (DoubleRow over the two F-halves) ----
            # xc group g holds 8 chunk-lhsTs contiguous per partition:
            # xc[g*128+p, ((j*2+a)*128)+m] = xperm.T[a*128+p, (g*8+j)*128+m]
            hs1 = hs1p.tile([128, kb, D], f8, tag="hs1")
            xr = xc.ap().rearrange("(g p) q -> g p q", p=128)
            xg = None
            for k4 in range(kb // 4):
                ph = pss.tile([128, 4, D], f32, tag="pss", name=f"ph1_{k4}")
                for i in range(4):
                    k = k4 * 4 + i
                    g, j = divmod(k, 8)
                    if j == 0:
                        xg = xcpool.tile([128, 8, 2, 128], f8, tag="xg")
                        (ringA if g % 2 == 0 else ringB).dma_start(
                            xg.rearrange("p j a m -> p (j a m)"), xr[g]
                        )
                    nc.tensor.matmul(
                        ph[:, i, :], xg[:, j, :, :], w1s[:],
                        start=(i == 0), stop=(i == 3), perf_mode=DR,
                    )
                nc.vector.tensor_copy(hs1[:, k4 * 4 : (k4 + 1) * 4, :], ph[:])

            ar0 = adjt0.ap().rearrange("(g p j) m -> g p (j m)", p=128, j=kg)
            ar1 = adjt1.ap().rearrange("(g p j) m -> g p (j m)", p=128, j=kg)

            def stream_pass(ar, apool, nch, inject):
                """One reversed-order DoubleRow streaming pass over `ar`.

                nch: list of (psum_tile, o, w) output chunks.
                inject: {group_iteration_index: callback} runs after that
                group's matmuls are emitted.
                Returns {g: sbuf tile}; entries g < adj_bufs stay resident.
                """
                tiles = {}
                kgp = kg // 2
                for gi, g in enumerate(reversed(range(ngrp))):
                    at = apool.tile([128, kg * half], f8, tag="a")
                    hw_ = kg * half // 2
                    ringA.dma_start(at[:, 0:hw_], ar[g][:, 0:hw_])
                    ringB.dma_start(at[:, hw_:], ar[g][:, hw_:])
                    tiles[g] = at
                    atv = at.rearrange("p (j m) -> p j m", j=kg)
                    for jp in range(kgp):
                        kpi = gi * kgp + jp
                        kp = g * kg + 2 * jp  # first k-block of the pair
                        for ps, o, w in nch:
                            nc.tensor.matmul(
                                ps[:],
                                hs1[:, kp : kp + 2, :],
                                atv[:, 2 * jp : 2 * jp + 2, o : o + w],
                                start=(kpi == 0),
                                stop=(kpi == kb // 2 - 1),
                                perf_mode=DR,
                            )
                    if gi in inject:
                        inject[gi]()
                return tiles

            def elu_z(ps, o, w, dst, corrap, i):
                # dst[:, o:o+w] = elu(ps/16 + corr), fp32
                z = tmp.tile([128, 512], f32, tag="z", name=f"z{i}")
                nc.vector.tensor_scalar(
                    z[:, :w], ps[:], 1.0 / 16.0, corrap, op0=OP.mult, op1=OP.add
                )
                neg = tmp.tile([128, 512], f32, tag="neg", name=f"neg{i}")
                nc.vector.tensor_scalar_min(neg[:, :w], z[:, :w], 0.0)
                ex = tmp.tile([128, 512], f32, tag="ex", name=f"ex{i}")
                nc.scalar.activation(ex[:, :w], neg[:, :w], AF.Exp)
                pm1 = tmp.tile([128, 512], f32, tag="pm1", name=f"pm1{i}")
                nc.vector.tensor_scalar(
                    pm1[:, :w], z[:, :w], 0.0, -1.0, op0=OP.max, op1=OP.add
                )
                nc.vector.tensor_add(dst[:, o : o + w], ex[:, :w], pm1[:, :w])

            x2t = xepool.tile([128, s], f32, tag="xe")
            cs2 = stat.tile([128, 2], f32, tag="cs2")
            hs2 = hs2p.tile([128, kb, D], f8, tag="hs2")
            x2hs = {}
            cshl = {}

            def half_soft(hx, psn):
                # elu + colsum partial (hi/lo e4m3) + fp16 cast, half hx
                def emit():
                    for i, (ps, o, w) in enumerate(psn):
                        elu_z(ps, hx * half + o, w, x2t, corr1s[:, 0:1], i)
                    nc.vector.tensor_reduce(
                        cs2[:, hx : hx + 1],
                        x2t[:, hx * half : (hx + 1) * half],
                        axis=mybir.AxisListType.X,
                        op=OP.add,
                    )
                    # hi/lo e4m3 encoding of colsum/2048 for the payload
                    t = stat.tile([128, 1], f32, tag=f"cst{hx}")
                    nc.vector.tensor_scalar_mul(t[:], cs2[:, hx : hx + 1], 1.0 / 2048.0)
                    hi8 = stat.tile([128, 1], f8, tag=f"hi8{hx}")
                    nc.vector.tensor_copy(hi8[:], t[:])
                    hi32 = stat.tile([128, 1], f32, tag=f"hi32{hx}")
                    nc.vector.tensor_copy(hi32[:], hi8[:])
                    lo = stat.tile([128, 1], f32, tag=f"lo{hx}")
                    nc.vector.tensor_sub(lo[:], t[:], hi32[:])
                    lo8 = stat.tile([128, 1], f8, tag=f"lo8{hx}")
                    nc.vector.tensor_scalar_mul(lo8[:], lo[:], 16.0)
                    cshl[hx] = (hi8, lo8)
                    x2h = xhpool.tile([128, half], f16, tag="xh", name=f"xh{hx}")
                    nc.vector.tensor_copy(x2h[:], x2t[:, hx * half : (hx + 1) * half])
                    x2hs[hx] = x2h

                return emit

            def half_hard(hx, h2b, h2f, slot0):
                # own-h2 (node-major) + bounce + AllGather + per-rank pulls
                def emit():
                    x2h = x2hs[hx]
                    h2o = h2opool.tile([128, c8, D], f8, tag="h2o", name=f"h2o{hx}")
                    for c in range(c8):
                        ph = pss.tile([128, D], f32, tag="pss", name=f"h2o_{hx}_{c}")
                        nc.tensor.matmul(
                            ph[:],
                            x2h[:, c * 128 : (c + 1) * 128],
                            w2s[:],
                            start=True,
                            stop=True,
                        )
                        nc.vector.tensor_copy(h2o[:, c, :], ph[:])
                    ringG.dma_start(h2b[:, 0 : c8 * 128], h2o[:])
                    hi8, lo8 = cshl[hx]
                    ringG.dma_start(h2b[:, c8 * 128 : c8 * 128 + 1], hi8[:])
                    ringG.dma_start(h2b[:, c8 * 128 + 1 : c8 * 128 + 2], lo8[:])
                    ringG.collective_compute(
                        "AllGather",
                        OP.bypass,
                        ins=[h2b.opt()],
                        outs=[h2f.opt()],
                        replica_groups=rg,
                    )
                    for r in range(ncores):
                        ringG.dma_start(
                            hs2[:, slot0 + r * c8 : slot0 + (r + 1) * c8, :],
                            h2f[r * 128 : (r + 1) * 128, 0 : c8 * 128],
                        )

                return emit

            # ---- layer 1, pass A (adj columns 0:half) ----
            psA = [
                (big.tile([128, w], f32, tag="big", name=f"pA{i}"), o, w)
                for i, (o, w) in enumerate(chunks(half))
            ]
            tilesA = stream_pass(ar0, ap0, psA, {})

            # ---- layer 1, pass B; half-A epilogue injected into its stream ----
            psB = [
                (big.tile([128, w], f32, tag="big", name=f"pB{i}"), o, w)
                for i, (o, w) in enumerate(chunks(half))
            ]
            tilesB = stream_pass(
                ar1, ap1, psB,
                {0: half_soft(0, psA),
                 min(1, ngrp - 1): half_hard(0, h2bA, h2fA, 0)},
            )

            # half-B elu must be emitted BEFORE layer 2 reuses psB's banks;
            # its gather goes out before the L2 stream so the PE-idle window
            # between the passes absorbs the h2o matmuls
            half_soft(1, psB)()
            half_hard(1, h2bB, h2fB, ncores * c8)()

            # ---- layer 2: full-width DoubleRow stream, forward order ----
            psL = [
                (big.tile([128, w], f32, tag="big", name=f"pL_{hx}_{i}"), hx, o, w)
                for hx in (0, 1)
                for i, (o, w) in enumerate(chunks(half))
            ]
            kgp = kg // 2
            for g in range(ngrp):
                if g < adj_bufs:
                    at0, at1 = tilesA[g], tilesB[g]
                else:
                    hw_ = kg * half // 2
                    at0 = ap0.tile([128, kg * half], f8, tag="a")
                    ringA.dma_start(at0[:, 0:hw_], ar0[g][:, 0:hw_])
                    ringB.dma_start(at0[:, hw_:], ar0[g][:, hw_:])
                    at1 = ap1.tile([128, kg * half], f8, tag="a")
                    ringA.dma_start(at1[:, 0:hw_], ar1[g][:, 0:hw_])
                    ringB.dma_start(at1[:, hw_:], ar1[g][:, hw_:])
                a0v = at0.rearrange("p (j m) -> p j m", j=kg)
                a1v = at1.rearrange("p (j m) -> p j m", j=kg)
                for jp in range(kgp):
                    kpi = g * kgp + jp
                    kp = g * kg + 2 * jp
                    for ps, hx, o, w in psL:
                        av = a0v if hx == 0 else a1v
                        nc.tensor.matmul(
                            ps[:],
                            hs2[:, kp : kp + 2, :],
                            av[:, 2 * jp : 2 * jp + 2, o : o + w],
                            start=(kpi == 0),
                            stop=(kpi == kb // 2 - 1),
                            perf_mode=DR,
                        )

            # corr2 = 0.5 * W2f.T @ colsum(x2): assemble from gathered hi/lo
            parts = stat.tile([128, 2, 2 * ncores], f8, tag="parts")
            for hx, h2f in ((0, h2fA), (1, h2fB)):
                hv = h2f.rearrange("(r p) q -> q p r", p=128)
                for t in (0, 1):
                    ringG.dma_start(
                        parts[:, t, hx * ncores : (hx + 1) * ncores],
                        hv[c8 * 128 + t],
                    )
            partsf = stat.tile([128, 2, 2 * ncores], f32, tag="partsf")
            nc.vector.tensor_copy(partsf[:], parts[:])
            ch = stat.tile([128, 2], f32, tag="ch")
            nc.vector.tensor_reduce(
                ch[:, 0:1], partsf[:, 0, :], axis=mybir.AxisListType.X, op=OP.add
            )
            nc.vector.tensor_reduce(
                ch[:, 1:2], partsf[:, 1, :], axis=mybir.AxisListType.X, op=OP.add
            )
            # colsum(x2) = 2048*CH + 128*CL = 128*(16*CH + CL)
            csum = stat.tile([128, 1], f32, tag="csum")
            nc.vector.tensor_scalar(
                csum[:], ch[:, 0:1], 16.0, ch[:, 1:2], op0=OP.mult, op1=OP.add
            )
            pcv = psy.tile([128, 1], f32, tag="psy", name="pcv")
            nc.tensor.matmul(pcv[:], w2s32[:], csum[:], start=True, stop=True)
            # corr2 = 0.5 * 128 * pcv
            corr2s = stat.tile([128, 1], f32, tag="corr2s")
            nc.vector.tensor_scalar_mul(corr2s[:], pcv[:], 64.0)

            # ---- pipelined tail: per-chunk elu -> out matmul -> softmax ----
            x3t = xepool.tile([128, s], f32, tag="xe")
            outr = out.ap().rearrange("(c p) m -> c p m", p=128)
            for i, (ps, hx, o, w) in enumerate(psL):
                off = hx * half + o
                elu_z(ps, off, w, x3t, corr2s[:, 0:1], i)
                crng = list(range(off // 128, (off + w) // 128))
                nchk = len(crng)
                zb = outp.tile([128, nchk, C], f32, tag="zb", name=f"zb{i}")
                for idx, c in enumerate(crng):
                    py = psy.tile([128, C], f32, tag="psy")
                    cs = slice(c * 128, (c + 1) * 128)
                    nc.tensor.matmul(
                        py[:], x3t[:, cs], wouts[:], start=True, stop=True
                    )
                    nc.vector.tensor_add(zb[:, idx, :], py[:], bbs[:])
                # elu over [128, nchk*C]
                zf = zb.rearrange("p c m -> p (c m)")
                negb = tmp.tile([128, nchk * C], f32, tag="neg", name=f"negb{i}")
                nc.vector.tensor_scalar_min(negb[:], zf, 0.0)
                eb = tmp.tile([128, nchk * C], f32, tag="ex", name=f"eb{i}")
                nc.scalar.activation(eb[:], negb[:], AF.Exp)
                pmb = tmp.tile([128, nchk * C], f32, tag="pm1", name=f"pmb{i}")
                nc.vector.tensor_scalar(
                    pmb[:], zf, 0.0, -1.0, op0=OP.max, op1=OP.add
                )
                zzb = outp.tile([128, nchk, C], f32, tag="zzb", name=f"zzb{i}")
                nc.vector.tensor_add(zzb.rearrange("p c m -> p (c m)"), eb[:], pmb[:])
                # log_softmax per 128-node chunk
                negm = stat.tile([128, nchk], f32, tag="negm", name=f"negm{i}")
                nc.vector.tensor_reduce(
                    negm[:], zzb[:], axis=mybir.AxisListType.X, op=OP.max, negate=True
                )
                ssum = stat.tile([128, nchk], f32, tag="ssum", name=f"ssum{i}")
                es = tmp.tile([128, nchk * C], f32, tag="z", name=f"es{i}")
                esv = es.rearrange("p (c m) -> p c m", m=C)
                for idx in range(nchk):
                    nc.scalar.activation(
                        esv[:, idx, :],
                        zzb[:, idx, :],
                        AF.Exp,
                        bias=negm[:, idx : idx + 1],
                        accum_out=ssum[:, idx : idx + 1],
                    )
                lse = stat.tile([128, nchk], f32, tag="lse", name=f"lse{i}")
                nc.scalar.activation(lse[:], ssum[:], AF.Ln)
                for idx, c in enumerate(crng):
                    osb = outp.tile([128, C], f32, tag="osb")
                    nc.vector.tensor_scalar(
                        osb[:],
                        zzb[:, idx, :],
                        negm[:, idx : idx + 1],
                        lse[:, idx : idx + 1],
                        op0=OP.add,
                        op1=OP.subtract,
                    )
                    ringG.dma_start(outr[c], osb[:])

    nc.compile()
    return nc


def make_in_maps(x, adj, W1, W2, Wout, bout, ncores=NCORES, kg=4):
    import ml_dtypes

    f8np = ml_dtypes.float8_e4m3
    n_total = adj.shape[0]
    s = n_total // ncores
    half = s // 2
    kb = n_total // 128
    ngrp = kb // kg
    kb8 = kb // 8
    f, d = W1.shape[1], W1.shape[0] * W1.shape[2]

    # global contraction-row order: all ranks' first halves, then seconds
    perm = np.concatenate(
        [np.arange(r * s, r * s + half) for r in range(ncores)]
        + [np.arange(r * s + half, (r + 1) * s) for r in range(ncores)]
    )

    w1f32 = np.ascontiguousarray(W1.transpose(1, 0, 2).reshape(f, d).astype(np.float32))
    w1f = w1f32.astype(f8np)
    w2f = np.ascontiguousarray(W2.transpose(1, 0, 2).reshape(d, d).astype(np.float16))
    woutf = np.ascontiguousarray(Wout.astype(np.float32))
    bbf = np.ascontiguousarray(
        np.broadcast_to(bout.astype(np.float32), (128, Wout.shape[1]))
    )
    # corr1 = 0.5 * colsum(h1) = 0.5 * colsum(x) @ W1f
    corr1 = (0.5 * (x.astype(np.float32).sum(0) @ w1f32)).astype(np.float32)
    corr1 = np.ascontiguousarray(corr1.reshape(d, 1))

    # fp8 e4m3 residual of adj, scaled by 16
    r8 = ((adj.astype(np.float32) - 0.5) * 16.0).astype(f8np)

    # xc[g*128 + p, ((j*2 + a)*128) + m] = xperm.T[a*128 + p, (g*8 + j)*128 + m]
    x8 = x.astype(f8np)
    xtc = x8[perm].T  # [F, n_total]
    xcf = np.ascontiguousarray(
        xtc.reshape(2, 128, kb8, 8, 128)
        .transpose(2, 1, 3, 0, 4)
        .reshape(kb8 * 128, 8 * f)
    )

    def reorder(t):
        # dram row (g*kg*128 + p*kg + j) <- k-row (g*kg*128 + j*128 + p)
        return np.ascontiguousarray(
            t.reshape(ngrp, kg, 128, t.shape[1]).transpose(0, 2, 1, 3).reshape(t.shape)
        )

    in_maps = []
    for c in range(ncores):
        t0 = r8[c * s : c * s + half, :][:, perm].T  # [n_total, half]
        t1 = r8[c * s + half : (c + 1) * s, :][:, perm].T
        in_maps.append(
            {
                "adjt0": reorder(t0),
                "adjt1": reorder(t1),
                "xc": xcf,
                "w1": w1f,
                "w2": w2f,
                "wout": woutf,
                "bb": bbf,
                "corr1": corr1,
            }
        )
    return in_maps


def kernel(x, adj, W1, W2, Wout, bout):
    from concourse import bass_utils

    x = np.asarray(x)
    adj = np.asarray(adj)
    in_maps = make_in_maps(x, adj, np.asarray(W1), np.asarray(W2),
                           np.asarray(Wout), np.asarray(bout))
    if "nc" not in _nc_cache:
        _nc_cache["nc"] = build_gat_nc()
    res = bass_utils.run_bass_kernel_spmd(
        _nc_cache["nc"], in_maps, core_ids=list(range(NCORES))
    )
    return np.concatenate([r["out"] for r in res.results], axis=0).astype(np.float32)
